# revision 1
# baseline (speedup 1.0000x reference)
"""AFNO (Adaptive Fourier Neural Operator) Trainium2 kernel.

Strategy: data-parallel over batch B=16 across 8 NeuronCores (2 samples/core,
no collectives). Per core, everything in bf16 on the TensorEngine:
  - bias path: out_bias = x @ bias_w.T + bias_b  (x^T built via PE
    identity-matmul transposes; bias_b folded in as a rank-1 matmul)
  - spectral path: rfft2 factored into two small real matmuls (W-axis rfft
    with stacked re/im output columns, then H-axis complex fft as a single
    stacked [64,64] matmul), block-diagonal complex MLP as stacked [384,384]
    matmuls (layer-2 "uses updated real part" composed into the weights on
    the host), then the inverse transforms. Layout changes between
    contraction axes ride DRAM bounce buffers with AP retargeting.
"""
import numpy as np

H, W, NB, C = 32, 64, 4, 768
KW = W // 2 + 1          # 33
BS = C // NB             # 192
B = 16
NCORES = 8
BL = B // NCORES         # 2 samples per core
N_TOK = H * W            # 2048
SLICES = BL * NB         # 8 per core: s = b*4 + blk

_CACHE = {}


def _transform_consts():
    w_idx = np.arange(W, dtype=np.float64)[:, None]
    kw_idx = np.arange(KW, dtype=np.float64)[None, :]
    th = 2 * np.pi * w_idx * kw_idx / W
    fw = np.concatenate([np.cos(th), -np.sin(th)], axis=1) / np.sqrt(W)    # [64, 66]

    h_idx = np.arange(H, dtype=np.float64)[:, None]
    kh_idx = np.arange(H, dtype=np.float64)[None, :]
    thh = 2 * np.pi * h_idx * kh_idx / H
    ch, sh = np.cos(thh), np.sin(thh)
    fh = np.zeros((64, 64))
    fh[0:32, 0:32] = ch
    fh[32:64, 0:32] = sh
    fh[0:32, 32:64] = -sh
    fh[32:64, 32:64] = ch
    fh /= np.sqrt(H)

    fhi = np.zeros((64, 64))
    fhi[0:32, 0:32] = ch.T
    fhi[32:64, 0:32] = -sh.T
    fhi[0:32, 32:64] = sh.T
    fhi[32:64, 32:64] = ch.T
    fhi /= np.sqrt(H)

    alpha = np.ones(KW)
    alpha[1:KW - 1] = 2.0
    thw = 2 * np.pi * kw_idx.T * np.arange(W, dtype=np.float64)[None, :] / W
    fwi = np.concatenate([alpha[:, None] * np.cos(thw),
                          -alpha[:, None] * np.sin(thw)], axis=0) / np.sqrt(W)  # [66, 64]

    # duplicate fw rows for the 2-slice row-packed W-fwd
    fwdup = np.concatenate([fw, fw], axis=0)  # [128, 66]
    return (fwdup.astype(np.float32), fh.astype(np.float32),
            fhi.astype(np.float32), fwi.astype(np.float32))


def _mlp_consts(w1, b1, w2, b2):
    w1 = np.asarray(w1, np.float64)
    b1 = np.asarray(b1, np.float64)
    w2 = np.asarray(w2, np.float64)
    b2 = np.asarray(b2, np.float64)
    m1 = np.zeros((NB, 2 * BS, 2 * BS))
    mb1 = np.zeros((NB, 2 * BS))
    m2 = np.zeros((NB, 2 * BS, 2 * BS))
    mb2 = np.zeros((NB, 2 * BS))
    for nb in range(NB):
        m1[nb, :BS, :BS] = w1[0, nb]
        m1[nb, BS:, :BS] = -w1[1, nb]
        m1[nb, :BS, BS:] = w1[1, nb]
        m1[nb, BS:, BS:] = w1[0, nb]
        mb1[nb, :BS] = b1[0, nb]
        mb1[nb, BS:] = b1[1, nb]
        A, Bm = w2[0, nb], w2[1, nb]
        m2[nb, :BS, :BS] = A
        m2[nb, BS:, :BS] = -Bm
        m2[nb, :BS, BS:] = A @ Bm
        m2[nb, BS:, BS:] = A - Bm @ Bm
        mb2[nb, :BS] = b2[0, nb]
        mb2[nb, BS:] = b2[0, nb] @ Bm + b2[1, nb]
    return (m1.astype(np.float32), mb1.astype(np.float32),
            m2.astype(np.float32), mb2.astype(np.float32))


def _build_graph():
    import concourse.bass as bass
    import concourse.mybir as mybir
    from concourse import bacc
    from concourse.tile import TileContext
    from concourse.tile_rust import add_dep_helper

    f32 = mybir.dt.float32
    bf16 = mybir.dt.bfloat16
    Relu = mybir.ActivationFunctionType.Relu
    CopyF = mybir.ActivationFunctionType.Identity
    ADD = mybir.AluOpType.add

    nc = bacc.Bacc()
    x_e = nc.declare_dram_parameter("x", [BL, N_TOK, C], f32, isOutput=False)
    fw_e = nc.declare_dram_parameter("fw", [128, 66], f32, isOutput=False)
    fh_e = nc.declare_dram_parameter("fh", [64, 64], f32, isOutput=False)
    fhi_e = nc.declare_dram_parameter("fhi", [64, 64], f32, isOutput=False)
    fwi_e = nc.declare_dram_parameter("fwi", [66, 64], f32, isOutput=False)
    m1_e = nc.declare_dram_parameter("m1", [NB, 2 * BS, 2 * BS], f32, isOutput=False)
    m2_e = nc.declare_dram_parameter("m2", [NB, 2 * BS, 2 * BS], f32, isOutput=False)
    mb1_e = nc.declare_dram_parameter("mb1", [NB, 2 * BS], f32, isOutput=False)
    mb2_e = nc.declare_dram_parameter("mb2", [NB, 2 * BS], f32, isOutput=False)
    wbt_e = nc.declare_dram_parameter("wbt", [C, C], f32, isOutput=False)
    bb_e = nc.declare_dram_parameter("bb", [1, C], f32, isOutput=False)
    ones_e = nc.declare_dram_parameter("ones", [1, 128], f32, isOutput=False)
    eye_e = nc.declare_dram_parameter("eye", [128, 128], f32, isOutput=False)
    out_e = nc.declare_dram_parameter("out", [BL, N_TOK, C], f32, isOutput=True)

    dA = nc.dram_tensor("dA", [SLICES, 2, KW, H, BS], bf16)   # (s, e, kw, h, cb)
    dC = nc.dram_tensor("dC", [SLICES, 2, BS, H, KW], bf16)
    dD = nc.dram_tensor("dD", [SLICES, 2, KW, H, BS], bf16)   # (s, e, kw, h, cb)

    FK = KW * BS   # 6336 spectral free size per slice (per 64-partition block)
    SK = H * BS    # 6144 spatial free size per slice

    # segments mapping (e, cb) -> (k chunk, p partition) for the MLP 384-row layout
    SEGS = [(0, 0, 0, 0, 128), (0, 128, 1, 0, 64), (1, 0, 1, 64, 64), (1, 64, 2, 0, 128)]
    # (e, cb0, k, p0, cnt)

    cnt = [0]

    def ccopy(dst, src):
        # alternate copy engine for load balance
        if cnt[0] % 2 == 0:
            nc.vector.tensor_copy(dst, src)
        else:
            nc.scalar.copy(dst, src)
        cnt[0] += 1

    with TileContext(nc) as tc:
        with (
            tc.tile_pool(name="const", bufs=1) as cpool,
            tc.tile_pool(name="big", bufs=1) as pool,
            tc.tile_pool(name="ps", bufs=8, space="PSUM") as psum,
        ):
            # ---- constants to SBUF (cast to bf16 via gpsimd DMA) ----
            fwsb = cpool.tile([128, 66], bf16)
            nc.gpsimd.dma_start(fwsb[:], fw_e.ap())
            fhsb = cpool.tile([64, 64], bf16)
            nc.gpsimd.dma_start(fhsb[:], fh_e.ap())
            fhisb = cpool.tile([64, 64], bf16)
            nc.gpsimd.dma_start(fhisb[:], fhi_e.ap())
            fwisb = cpool.tile([66, 64], bf16)
            nc.gpsimd.dma_start(fwisb[:], fwi_e.ap())
            m1sb = cpool.tile([128, 16, 2 * BS], bf16)
            m2sb = cpool.tile([128, 12, 2 * BS], bf16)
            ROWCH = [(0, 128), (128, 64), (192, 128), (320, 64)]
            for blk in range(NB):
                for j, (r0, rn) in enumerate(ROWCH):
                    nc.gpsimd.dma_start(m1sb[0:rn, blk * 4 + j, :],
                                        m1_e.ap()[blk, r0:r0 + rn, :])
                for k in range(3):
                    nc.gpsimd.dma_start(m2sb[:, blk * 3 + k, :],
                                        m2_e.ap()[blk, k * 128:(k + 1) * 128, :])
            mb1sb = cpool.tile([128, 12], f32)
            mb2sb = cpool.tile([128, 12], f32)
            for blk in range(NB):
                for m in range(3):
                    nc.gpsimd.dma_start(
                        mb1sb[:, blk * 3 + m:blk * 3 + m + 1],
                        mb1_e.ap()[blk, m * 128:(m + 1) * 128].rearrange("(a b) -> a b", b=1))
                    nc.gpsimd.dma_start(
                        mb2sb[:, blk * 3 + m:blk * 3 + m + 1],
                        mb2_e.ap()[blk, m * 128:(m + 1) * 128].rearrange("(a b) -> a b", b=1))
            wbtsb = cpool.tile([128, 6, C], bf16)
            for k in range(6):
                nc.gpsimd.dma_start(wbtsb[:, k, :], wbt_e.ap()[k * 128:(k + 1) * 128, :])
            bbsb = cpool.tile([1, C], bf16)
            nc.gpsimd.dma_start(bbsb[:], bb_e.ap())
            onesb = cpool.tile([1, 128], bf16)
            nc.gpsimd.dma_start(onesb[:], ones_e.ap())
            eyesb = cpool.tile([128, 128], bf16)
            nc.gpsimd.dma_start(eyesb[:], eye_e.ap())

            bias_stores = {0: [], 1: []}

            def fft_pair(p):
                """Spectral path for slice pair (2p, 2p+1)."""
                s0 = 2 * p
                # load both slices: [w | h, cb] rows j*64..
                X1 = pool.tile([128, SK], bf16)
                for j in range(2):
                    s = s0 + j
                    b, blk = s // 4, s % 4
                    src = x_e.ap()[b, :, blk * BS:(blk + 1) * BS].rearrange(
                        "(h w) c -> w h c", w=W)
                    nc.gpsimd.dma_start(X1[j * 64:(j + 1) * 64, :].rearrange(
                        "w (h c) -> w h c", c=BS), src)  # stays SWDGE (casts)

                # W-fwd (row-packed pair) -> V1[j] [66 | h, cb]
                V1 = [pool.tile([66, SK], bf16, tag=f"V1_{j}", name=f"V1_{j}") for j in range(2)]
                for n in range(12):
                    for j in range(2):
                        ps = psum.tile([66, 512], mybir.dt.float32, tag="ps")
                        nc.tensor.matmul(ps[:], fwsb[j * 64:(j + 1) * 64, :],
                                         X1[j * 64:(j + 1) * 64, n * 512:(n + 1) * 512],
                                         start=True, stop=True,
                                         tile_position=(j * 64, 0))
                        ccopy(V1[j][:, n * 512:(n + 1) * 512], ps[:])

                for j in range(2):
                    s = s0 + j
                    b, blk = s // 4, s % 4
                    # bounce A write: dA[s] = [e][kw][h][cb]
                    wA = nc.sync.dma_start(
                        dA.ap()[s].rearrange("e kw h c -> (e kw) (h c)"), V1[j][:])
                    # bounce A read -> V2 [64=(e,h) | kw, cb]
                    V2 = pool.tile([64, FK], bf16, tag="V2", bufs=2)
                    for e in range(2):
                        rA = nc.gpsimd.dma_start(
                            V2[e * 32:(e + 1) * 32, :].rearrange("h (kw c) -> h kw c", c=BS),
                            dA.ap()[s, e].rearrange("kw h c -> h kw c"))
                        add_dep_helper(rA.ins, wA.ins, reason="dA RAW")

                    # H-fwd (data stationary): lhsT = V2-slice [64, cb-chunk],
                    # rhs = FH -> out psum [cb-chunk, (kw-batch, e', kh)].
                    # Output partitions = cb-chunk == the MLP's K-chunk rows.
                    MLPin = [pool.tile([128, 2, H * KW], bf16, tag=f"MLPin{e2}",
                                       name=f"MLPin{e2}", bufs=2) for e2 in range(2)]
                    for cc in range(2):
                        rows = 128 if cc == 0 else 64
                        for g in range(5):
                            kw0 = g * 8
                            nkw = min(33 - kw0, 8)
                            ps = psum.tile([128, 512], mybir.dt.float32, tag="ps",
                                           name="psHf")
                            for i in range(nkw):
                                kw = kw0 + i
                                nc.tensor.matmul(
                                    ps[0:rows, i * 64:(i + 1) * 64],
                                    V2[:, kw * BS + cc * 128: kw * BS + cc * 128 + rows],
                                    fhsb[:], start=True, stop=True)
                            srcv = ps[0:rows, 0:nkw * 64].rearrange(
                                "p (kw e kh) -> p e kw kh", e=2, kh=32)
                            for e2 in range(2):
                                dstv = MLPin[e2][0:rows, cc, :].rearrange(
                                    "p (kh kw) -> p kw kh", kw=KW)[:, kw0:kw0 + nkw, :]
                                ccopy(dstv, srcv[:, e2])

                    # MLP layers
                    TT = H * KW  # 1056
                    KCH = [(0, 0, 128), (0, 1, 64), (1, 0, 128), (1, 1, 64)]
                    Z1 = pool.tile([128, 3, TT], bf16, tag="Z1", bufs=2)
                    for m in range(3):
                        for t3 in range(3):
                            ps = psum.tile([128, 352], mybir.dt.float32, tag="ps")
                            for i, (e2, cc, rows) in enumerate(KCH):
                                nc.tensor.matmul(
                                    ps[:], m1sb[0:rows, blk * 4 + 2 * e2 + cc,
                                                m * 128:(m + 1) * 128],
                                    MLPin[e2][0:rows, cc, t3 * 352:(t3 + 1) * 352],
                                    start=(i == 0), stop=(i == 3))
                            nc.scalar.activation(Z1[:, m, t3 * 352:(t3 + 1) * 352],
                                                 ps[:], Relu,
                                                 bias=mb1sb[:, blk * 3 + m:blk * 3 + m + 1])
                    Z2 = pool.tile([128, 3, TT], bf16, tag="Z2", bufs=2)
                    for m in range(3):
                        for t3 in range(3):
                            ps = psum.tile([128, 352], mybir.dt.float32, tag="ps")
                            for k in range(3):
                                nc.tensor.matmul(
                                    ps[:], m2sb[:, blk * 3 + k, m * 128:(m + 1) * 128],
                                    Z1[:, k, t3 * 352:(t3 + 1) * 352],
                                    start=(k == 0), stop=(k == 2))
                            nc.scalar.activation(Z2[:, m, t3 * 352:(t3 + 1) * 352],
                                                 ps[:], CopyF,
                                                 bias=mb2sb[:, blk * 3 + m:blk * 3 + m + 1])

                    # bounce C write (mirror of B read)
                    wC = []
                    for e, cb0, k, p0, seg in SEGS:
                        wC.append((e, nc.sync.dma_start(
                            dC.ap()[s, e, cb0:cb0 + seg].rearrange("c kh kw -> c (kh kw)"),
                            Z2[p0:p0 + seg, k, :])))
                    # bounce C read -> V4 [64=(e,kh) | cb, kw]
                    V4 = pool.tile([64, FK], bf16, tag="V4", bufs=2)
                    for e in range(2):
                        dmaeng = nc.sync if e == 0 else nc.gpsimd
                        rC = dmaeng.dma_start(
                            V4[e * 32:(e + 1) * 32, :].rearrange("p (c kw) -> p c kw", kw=KW),
                            dC.ap()[s, e].rearrange("c kh kw -> kh c kw"))
                        for we, wi in wC:
                            if we == e:
                                add_dep_helper(rC.ins, wi.ins, reason="dC RAW")

                    # H-inv: chunks of 330 (10 cb) x19 + tail 66; reorder copy to
                    # V5 [64=(e,h) | kw, cb]  (off = kw*192 + cb)
                    V5 = pool.tile([64, FK], bf16, tag="V5")
                    v5c = V5[:].rearrange("p (kw c) -> p c kw", c=BS)
                    for n in range(20):
                        NN = 330 if n < 19 else 66
                        ncb = 10 if n < 19 else 2
                        ps = psum.tile([64, 330], mybir.dt.float32, tag="ps")
                        nc.tensor.matmul(ps[:, 0:NN], fhisb[:],
                                         V4[:, n * 330:n * 330 + NN],
                                         start=True, stop=True)
                        ccopy(v5c[:, n * 10:n * 10 + ncb, :],
                              ps[:, 0:NN].rearrange("p (c kw) -> p c kw", kw=KW))

                    # bounce D write: dD[s] = [e][kw][h][cb]
                    wD = []
                    for e in range(2):
                        wD.append(nc.sync.dma_start(
                            dD.ap()[s, e].rearrange("kw h c -> h kw c"),
                            V5[e * 32:(e + 1) * 32, :].rearrange("p (kw c) -> p kw c", c=BS)))
                    # bounce D read -> V6 [66=(e,kw) | h, cb]
                    V6 = pool.tile([66, SK], bf16, tag="V6")
                    for e in range(2):
                        rD = nc.gpsimd.dma_start(
                            V6[e * KW:(e + 1) * KW, :].rearrange("p (h c) -> p h c", c=BS),
                            dD.ap()[s, e])
                        add_dep_helper(rD.ins, wD[e].ins, reason="dD RAW")

                    # W-inv -> V7 [w=64 | h, cb] (real)
                    V7 = pool.tile([64, SK], bf16, tag="V7")
                    for n in range(12):
                        ps = psum.tile([64, 512], mybir.dt.float32, tag="ps")
                        nc.tensor.matmul(ps[:], fwisb[:], V6[:, n * 512:(n + 1) * 512],
                                         start=True, stop=True)
                        ccopy(V7[:, n * 512:(n + 1) * 512], ps[:])

                    # accumulate the spectral-path result onto the bias
                    # result already stored in out_e (DMA accum, bf16 -> f32)
                    aE = nc.gpsimd.dma_start(
                        out_e.ap()[b, :, blk * BS:(blk + 1) * BS].rearrange(
                            "(h w) c -> w h c", w=W),
                        V7[:].rearrange("p (h c) -> p h c", c=BS),
                        accum_op=ADD)
                    for st in bias_stores[b]:
                        add_dep_helper(aE.ins, st.ins, reason="bias before accum")

            def bias_tiles(b):
                """bias matmul + final add + store for sample b (16 t-tiles)."""
                for j in range(16):
                    r0 = j * 128
                    xnat = pool.tile([128, C], bf16, tag="xnat", bufs=2)
                    nc.gpsimd.dma_start(xnat[:], x_e.ap()[b, r0:r0 + 128, :])
                    XTsb = pool.tile([128, C], bf16, tag="XTsb", bufs=2)
                    for half in range(2):
                        pst = psum.tile([128, 384], mybir.dt.float32, tag="ps")
                        for cc in range(3):
                            col = half * 3 + cc
                            nc.tensor.matmul(pst[:, cc * 128:(cc + 1) * 128],
                                             xnat[:, col * 128:(col + 1) * 128],
                                             eyesb[:], start=True, stop=True)
                        ccopy(XTsb[:, half * 384:(half + 1) * 384], pst[:])
                    OUT = pool.tile([128, C], mybir.dt.float32, tag="OUT", bufs=2)
                    for co2 in range(2):
                        bp = psum.tile([128, 384], mybir.dt.float32, tag="ps")
                        for k in range(6):
                            nc.tensor.matmul(bp[:], XTsb[:, k * 128:(k + 1) * 128],
                                             wbtsb[:, k, co2 * 384:(co2 + 1) * 384],
                                             start=(k == 0), stop=False)
                        nc.tensor.matmul(bp[:], onesb[:],
                                         bbsb[:, co2 * 384:(co2 + 1) * 384],
                                         start=False, stop=True)
                        ccopy(OUT[:, co2 * 384:(co2 + 1) * 384], bp[:])
                    st = nc.sync.dma_start(out_e.ap()[b, r0:r0 + 128, :], OUT[:])
                    bias_stores[b].append(st)

            bias_tiles(0)
            bias_tiles(1)
            fft_pair(0)
            fft_pair(1)
            fft_pair(2)
            fft_pair(3)

    try:
        ents = tc._perfetto_entries
        mx = max((e[2] for e in ents if e[2] is not None), default=0)
        mn = min((e[1] for e in ents if e[1] is not None), default=0)
        print(f"[tile-sim] predicted makespan ~= {(mx - mn) / 1000.0:.1f} us")
    except Exception as ex:
        print("[tile-sim] no makespan:", ex)
    nc.compile()
    return nc


def kernel(x, w1, b1, w2, b2, bias_w, bias_b):
    from concourse.bass_utils import run_bass_kernel_spmd

    if "nc" not in _CACHE:
        _CACHE["nc"] = _build_graph()
    nc = _CACHE["nc"]

    fw, fh, fhi, fwi = _transform_consts()
    m1, mb1, m2, mb2 = _mlp_consts(w1, b1, w2, b2)
    x = np.ascontiguousarray(np.asarray(x, np.float32))
    wbt = np.ascontiguousarray(np.asarray(bias_w, np.float32).T)
    bb = np.asarray(bias_b, np.float32).reshape(1, C)
    ones = np.ones((1, 128), np.float32)
    eye = np.eye(128, dtype=np.float32)

    base = dict(fw=fw, fh=fh, fhi=fhi, fwi=fwi, m1=m1, m2=m2, mb1=mb1, mb2=mb2,
                wbt=wbt, bb=bb, ones=ones, eye=eye)
    in_maps = []
    for i in range(NCORES):
        m = dict(base)
        m["x"] = np.ascontiguousarray(x[i * BL:(i + 1) * BL])
        in_maps.append(m)

    res = run_bass_kernel_spmd(nc, in_maps, list(range(NCORES)))
    out = np.concatenate([r["out"] for r in res.results], axis=0)
    return out.astype(np.float32)

